# revision 13
# baseline (speedup 1.0000x reference)
"""AWBNet (wo R2) Trainium2 kernel, v2.

Math (per sample b):
  m = reshape(relu(hist_flat @ W1 + b1) @ W2 + b2, [9, 3])
  y[px, c] = m0c r + m1c g + m2c b + m3c r^2 + m4c g^2 + m5c b^2
           + m6c rg + m7c rb + m8c gb

Device strategy (8 cores, pure data parallel, 2 samples/core):
  * MLP: the full W1 is streamed per core as fp16 (host-cast; the device
    DMA would cast to fp16 anyway, this just halves the HBM read) on the
    sync HWDGE ring, in chunks pipelined with the 96 accumulating PE
    matmuls (lhsT = packed histogram slices [128, 2]).  feat -> relu ->
    PE transpose -> stride-0-broadcast W2 matmul produce mscal[P, 54]
    fp32 coefficients replicated across partitions.
  * Pixels: one tile per sample, [128, 2048] planar fp16 planes loaded
    via SWDGE cast DMAs (host pre-packs x planar, so there is no on-device
    deinterleave).  Per-pixel evaluation uses the Horner form
        y_c = R*(a0 + a3 R + a6 G + a7 B) + G*(a1 + a4 G + a8 B)
            + B*(a2 + a5 B)
    with the per-channel scalar products on ACT (scale/bias activations)
    and DVE (4x-mode tensor_scalar), and all tensor-tensor combines as
    channel-merged wide [128, 3, 2048] DVE ops (2x fp16 mode).  The Pool
    engine is deliberately compute-free: its Q7 tensor ops are slow and
    degrade concurrent DVE throughput (measured), so it only issues the
    SWDGE cast DMAs.
  * y is produced as fp16 planes and stored fp16 (half the write
    traffic); the host casts back to fp32 on assembly.
"""

import sys

import numpy as np

for _p in ("/opt/trn_rl_repo",):
    if _p not in sys.path:
        sys.path.insert(0, _p)

import concourse.bacc as bacc
import concourse.mybir as mybir
import concourse.tile as tile
from concourse import bass_utils

# ---- problem constants (hardcoded per contract) ----
N_CORES = 8
B, H, W, C = 16, 512, 512, 3
SPC = B // N_CORES  # samples per core = 2
PX_SAMPLE = H * W  # 262144
P = 128
T = PX_SAMPLE // P  # 2048 pixels per partition; one tile per sample
NT = SPC  # 2 tiles per core

HIST = 3 * 64 * 64  # 12288
HID = 256
MOUT = 27
KT = HIST // P  # 96 k-tiles
GP = 2  # k-tiles packed per PE matmul (lhsT [128, 4], rhs [128, 512])
NGRP = KT // GP  # 24 matmul groups
CH_G = 2  # groups per W1 chunk DMA (4KB/partition contiguous)
NCH = NGRP // CH_G  # 12 chunks

F16 = mybir.dt.float16
F32 = mybir.dt.float32
MULT = mybir.AluOpType.mult
ADD = mybir.AluOpType.add
AF = mybir.ActivationFunctionType

_CACHE = {}


def _build():
    nc = bacc.Bacc(
        "TRN2", target_bir_lowering=False, debug=False, num_devices=N_CORES
    )

    # planar pixel input [tile(=sample), ch, part, T]
    x_d = nc.dram_tensor("x_core", [NT, C, P, T], F32, kind="ExternalInput")
    # histogram for this core's 2 samples, packed [k2, gg*(2*GP) + 2j + s]
    hp_d = nc.dram_tensor("h_packed", [P, KT * SPC], F32, kind="ExternalInput")
    # full W1, host-cast fp16, group-packed [k2, gg, 256j + n]
    w1_d = nc.dram_tensor("w1h", [P, NGRP, GP * HID], F16, kind="ExternalInput")
    b1_d = nc.dram_tensor("b1_rep", [SPC, HID], F32, kind="ExternalInput")
    w2_d = nc.dram_tensor("w2p", [HID // P, P, MOUT], F32, kind="ExternalInput")
    b2_d = nc.dram_tensor("b2bc", [P, SPC * MOUT], F32, kind="ExternalInput")
    eye_d = nc.dram_tensor("eye2", [SPC, SPC], F32, kind="ExternalInput")
    y_d = nc.dram_tensor("y_core", [NT, C, P, T], F16, kind="ExternalOutput")

    MT = HID // P  # 2

    with tile.TileContext(nc) as tc:
        with (
            tc.tile_pool(name="mlp", bufs=1) as mlp_pool,
            tc.tile_pool(name="w1s", bufs=2) as w1_pool,
            tc.tile_pool(name="xin", bufs=2) as x_pool,
            tc.tile_pool(name="pla", bufs=2) as pa_pool,
            tc.tile_pool(name="plb", bufs=1) as pb_pool,
            tc.tile_pool(name="yout", bufs=1) as y_pool,
            tc.tile_pool(name="ps", bufs=1, space="PSUM") as psum_pool,
        ):
            # ---------------- MLP ----------------
            hp_sb = mlp_pool.tile([P, KT * SPC], F16, tag="hp", name="hp")
            nc.gpsimd.dma_start(out=hp_sb, in_=hp_d[:, :])

            b1_sb = mlp_pool.tile([SPC, HID], F32, tag="b1", name="b1")
            nc.scalar.dma_start(out=b1_sb, in_=b1_d[:, :])
            w2_sb = mlp_pool.tile([P, MT, MOUT], F32, tag="w2", name="w2")
            nc.scalar.dma_start(out=w2_sb, in_=w2_d.rearrange("m p n -> p m n"))
            b2_sb = mlp_pool.tile([P, SPC * MOUT], F32, tag="b2", name="b2")
            nc.scalar.dma_start(out=b2_sb, in_=b2_d[:, :])
            eye_sb = mlp_pool.tile([SPC, SPC], F32, tag="eye", name="eye")
            nc.scalar.dma_start(out=eye_sb, in_=eye_d[:, :])

            # 4-packed accumulating matmuls: lhsT [128, 8] covers 4 k-tiles x
            # 2 samples; rhs [128, 1024] = the 4 k-tiles' W1 side by side.
            # psum row 2j+s, cols [256j, 256j+256) holds sample s's partial
            # from k-tile subset j (other cells accumulate don't-care data).
            feat_ps = psum_pool.tile([SPC * GP, GP * HID], F32, tag="featps", name="featps")
            for ci in range(NCH):
                w1c = w1_pool.tile([P, CH_G, GP * HID], F16, tag="w1c", name=f"w1c{ci}")
                nc.gpsimd.dma_start(
                    out=w1c, in_=w1_d[:, ci * CH_G : (ci + 1) * CH_G, :]
                )
                for gi in range(CH_G):
                    gg = ci * CH_G + gi
                    nc.tensor.matmul(
                        feat_ps,
                        hp_sb[:, gg * SPC * GP : (gg + 1) * SPC * GP],
                        w1c[:, gi, :],
                        start=(gg == 0),
                        stop=(gg == NGRP - 1),
                    )
            # fold the GP k-tile subsets: [4, 512] -> [2, 256].  Engine reads
            # must start at 32-aligned partitions, so stage the psum in SBUF
            # and move the odd half down with a small SBUF->SBUF DMA.
            u0 = mlp_pool.tile([SPC * GP, GP * HID], F32, tag="u0", name="u0")
            nc.vector.tensor_copy(u0, feat_ps)
            uB = mlp_pool.tile([SPC, HID], F32, tag="uB", name="uB")
            nc.sync.dma_start(out=uB, in_=u0[SPC : 2 * SPC, HID : 2 * HID])
            u2 = mlp_pool.tile([SPC, HID], F32, tag="u2", name="u2")
            nc.vector.tensor_add(u2, u0[0:SPC, 0:HID], uB)
            feat_b = mlp_pool.tile([SPC, HID], F32, tag="featb", name="featb")
            nc.vector.tensor_add(feat_b, u2, b1_sb)
            feat_r = mlp_pool.tile([SPC, HID], F32, tag="featr", name="featr")
            nc.vector.tensor_scalar(feat_r, feat_b, 0.0, None, mybir.AluOpType.max)

            featT_sb = []
            for mt in range(MT):
                ft_ps = psum_pool.tile([P, SPC], F32, tag=f"ftps{mt}", name=f"ftps{mt}")
                nc.tensor.transpose(ft_ps, feat_r[:, mt * P : (mt + 1) * P], eye_sb)
                ft_sb = mlp_pool.tile([P, SPC], F32, tag=f"ft{mt}", name=f"ft{mt}")
                nc.vector.tensor_copy(ft_sb, ft_ps)
                featT_sb.append(ft_sb)

            mb_ps = psum_pool.tile([P, SPC * MOUT], F32, tag="mbps", name="mbps")
            for s in range(SPC):
                for mt in range(MT):
                    nc.tensor.matmul(
                        mb_ps[:, s * MOUT : (s + 1) * MOUT],
                        featT_sb[mt][:, s : s + 1].broadcast_to([P, P]),
                        w2_sb[:, mt, :],
                        start=(mt == 0),
                        stop=(mt == MT - 1),
                    )
            mscal = mlp_pool.tile([P, SPC * MOUT], F32, tag="mscal", name="mscal")
            nc.vector.tensor_add(mscal, mb_ps, b2_sb)

            # ---------------- pixel path (Horner) ----------------
            for t in range(NT):
                def ms(k, c, s=t):
                    j = s * MOUT + 3 * k + c
                    return mscal[:, j : j + 1]

                xt = x_pool.tile([P, C, T], F16, tag="xt", name=f"xt{t}")
                for c in range(C):
                    nc.gpsimd.dma_start(out=xt[:, c, :], in_=x_d[t, c])
                R, G, Bp = xt[:, 0, :], xt[:, 1, :], xt[:, 2, :]
                Rw = xt[:, 0:1, :].broadcast_to([P, C, T])
                Gw = xt[:, 1:2, :].broadcast_to([P, C, T])
                Bw = xt[:, 2:3, :].broadcast_to([P, C, T])

                ysb = y_pool.tile([P, C, T], F16, tag="ysb", name=f"ysb{t}")

                # per-channel scalar products into channel slices of wide
                # tiles.  DVE (4x tensor_scalar): a1 = a3*R + a0,
                # b1 = a4*G + a1, cc = a5*B + a2; ACT: the pure muls,
                # role-grouped so the wide combines unblock early.
                a1w = pa_pool.tile([P, C, T], F16, tag="a1w", name=f"a1w{t}")
                a2w = pa_pool.tile([P, C, T], F16, tag="a2w", name=f"a2w{t}")
                a3w = pa_pool.tile([P, C, T], F16, tag="a3w", name=f"a3w{t}")
                b1w = pa_pool.tile([P, C, T], F16, tag="b1w", name=f"b1w{t}")
                b2w = pb_pool.tile([P, C, T], F16, tag="b2w", name=f"b2w{t}")
                ccw = pb_pool.tile([P, C, T], F16, tag="ccw", name=f"ccw{t}")
                for c in range(C):
                    nc.vector.tensor_scalar(
                        a1w[:, c, :], R, ms(3, c), ms(0, c), MULT, ADD
                    )
                for c in range(C):
                    nc.vector.tensor_scalar(
                        b1w[:, c, :], G, ms(4, c), ms(1, c), MULT, ADD
                    )
                for c in range(C):
                    nc.vector.tensor_scalar(
                        ccw[:, c, :], Bp, ms(5, c), ms(2, c), MULT, ADD
                    )
                for c in range(C):
                    nc.scalar.mul(a2w[:, c, :], G, ms(6, c))
                for c in range(C):
                    nc.scalar.mul(a3w[:, c, :], Bp, ms(7, c))
                for c in range(C):
                    nc.scalar.mul(b2w[:, c, :], Bp, ms(8, c))

                # wide channel-merged combines on DVE
                a12 = pb_pool.tile([P, C, T], F16, tag="a12", name=f"a12{t}")
                nc.vector.tensor_add(a12, a1w, a2w)
                aa = pb_pool.tile([P, C, T], F16, tag="aa", name=f"aa{t}")
                nc.vector.tensor_add(aa, a12, a3w)
                ra = pa_pool.tile([P, C, T], F16, tag="a2w", name=f"ra{t}")
                nc.vector.tensor_mul(ra, Rw, aa)
                bc = pa_pool.tile([P, C, T], F16, tag="a1w", name=f"bc{t}")
                nc.vector.tensor_mul(bc, Bw, ccw)
                bb = pb_pool.tile([P, C, T], F16, tag="bb", name=f"bb{t}")
                nc.vector.tensor_add(bb, b1w, b2w)
                gb = pa_pool.tile([P, C, T], F16, tag="a3w", name=f"gb{t}")
                nc.vector.tensor_mul(gb, Gw, bb)
                y1 = pa_pool.tile([P, C, T], F16, tag="b1w", name=f"y1_{t}")
                nc.vector.tensor_add(y1, ra, gb)
                nc.vector.tensor_add(ysb, y1, bc)

                nc.sync.dma_start(out=y_d[t].rearrange("c p j -> p c j"), in_=ysb)

    nc.compile()
    return nc


def _prep_inputs(x, histogram, W1, b1, W2, b2):
    """Host-side sharding / layout packing.  The only host dtype change is
    W1 fp32->fp16 (identical values to what the device cast DMA would
    produce; halves the streamed bytes)."""
    x = np.asarray(x, dtype=np.float32)
    hist = np.asarray(histogram, dtype=np.float32).reshape(B, HIST)
    W1 = np.asarray(W1, dtype=np.float32)
    b1 = np.asarray(b1, dtype=np.float32)
    W2 = np.asarray(W2, dtype=np.float32)
    b2 = np.asarray(b2, dtype=np.float32)

    # [k, n] -> [k2, gg, 256j + n] fp16 (4 k-tiles packed side by side)
    w1h = np.ascontiguousarray(
        W1.reshape(NGRP, GP, P, HID)
        .transpose(2, 0, 1, 3)
        .reshape(P, NGRP, GP * HID)
        .astype(np.float16)
    )
    w2p = np.ascontiguousarray(W2.reshape(HID // P, P, MOUT))
    b1rep = np.ascontiguousarray(np.broadcast_to(b1, (SPC, HID)))
    b2bc = np.ascontiguousarray(np.broadcast_to(np.tile(b2, SPC), (P, SPC * MOUT)))
    eye2 = np.eye(SPC, dtype=np.float32)

    in_maps = []
    for core in range(N_CORES):
        # pixels of sample s: [px, ch] -> [ch, p, j], px = p*T + j
        xs = x[core * SPC : (core + 1) * SPC].reshape(SPC, P, T, C)
        x_core = np.ascontiguousarray(xs.transpose(0, 3, 1, 2))
        hs = hist[core * SPC : (core + 1) * SPC]  # [2, HIST]
        # hp[k2, gg*8 + 2j + s] = h[s, (4gg+j)*128 + k2]
        hp = np.ascontiguousarray(
            hs.reshape(SPC, NGRP, GP, P)
            .transpose(3, 1, 2, 0)
            .reshape(P, KT * SPC)
        )
        in_maps.append(
            {
                "x_core": x_core,
                "h_packed": hp,
                "w1h": w1h,
                "b1_rep": b1rep,
                "w2p": w2p,
                "b2bc": b2bc,
                "eye2": eye2,
            }
        )
    return in_maps


def run(trace=False, **inputs):
    if "nc" not in _CACHE:
        _CACHE["nc"] = _build()
    nc = _CACHE["nc"]
    in_maps = _prep_inputs(**inputs)
    res = bass_utils.run_bass_kernel_spmd(
        nc, in_maps, core_ids=list(range(N_CORES)), trace=trace
    )
    outs = np.stack([r["y_core"] for r in res.results])  # [8, NT, C, P, T] f16
    # [core, s, c, p, j] -> [B, H, W, C]
    y = (
        outs.reshape(N_CORES * SPC, C, P * T)
        .transpose(0, 2, 1)
        .reshape(B, H, W, C)
        .astype(np.float32)
    )
    return y, res


def kernel(**inputs) -> np.ndarray:
    y, _ = run(trace=False, **inputs)
    return y


if __name__ == "__main__":
    rng = np.random.default_rng(0)
    ins = {
        "x": rng.random((B, H, W, C), dtype=np.float32),
        "histogram": rng.random((B, 3, 64, 64), dtype=np.float32),
        "W1": (rng.standard_normal((HIST, HID)) / np.sqrt(HIST)).astype(np.float32),
        "b1": np.zeros(HID, np.float32),
        "W2": (rng.standard_normal((HID, MOUT)) / np.sqrt(HID)).astype(np.float32),
        "b2": np.zeros(MOUT, np.float32),
    }
    y = kernel(**ins)
    print("out", y.shape, y.dtype, float(np.abs(y).max()))


# revision 17
# speedup vs baseline: 1.1128x; 1.1128x over previous
"""AWBNet (wo R2) Trainium2 kernel, v2.

Math (per sample b):
  m = reshape(relu(hist_flat @ W1 + b1) @ W2 + b2, [9, 3])
  y[px, c] = m0c r + m1c g + m2c b + m3c r^2 + m4c g^2 + m5c b^2
           + m6c rg + m7c rb + m8c gb

Device strategy (8 cores, pure data parallel, 2 samples/core):
  * MLP: the full W1 is streamed per core as fp16 (host-cast; the device
    DMA would cast to fp16 anyway, this just halves the HBM read) on the
    sync HWDGE ring, in chunks pipelined with the 96 accumulating PE
    matmuls (lhsT = packed histogram slices [128, 2]).  feat -> relu ->
    PE transpose -> stride-0-broadcast W2 matmul produce mscal[P, 54]
    fp32 coefficients replicated across partitions.
  * Pixels: one tile per sample, [128, 2048] planar fp16 planes loaded
    via SWDGE cast DMAs (host pre-packs x planar, so there is no on-device
    deinterleave).  Per-pixel evaluation uses the Horner form
        y_c = R*(a0 + a3 R + a6 G + a7 B) + G*(a1 + a4 G + a8 B)
            + B*(a2 + a5 B)
    with the per-channel scalar products on ACT (scale/bias activations)
    and DVE (4x-mode tensor_scalar), and all tensor-tensor combines as
    channel-merged wide [128, 3, 2048] DVE ops (2x fp16 mode).  The Pool
    engine is deliberately compute-free: its Q7 tensor ops are slow and
    degrade concurrent DVE throughput (measured), so it only issues the
    SWDGE cast DMAs.
  * y is produced as fp16 planes and stored fp16 (half the write
    traffic); the host casts back to fp32 on assembly.
"""

import sys

import numpy as np

for _p in ("/opt/trn_rl_repo",):
    if _p not in sys.path:
        sys.path.insert(0, _p)

import concourse.bacc as bacc
import concourse.mybir as mybir
import concourse.tile as tile
from concourse import bass_utils

# ---- problem constants (hardcoded per contract) ----
N_CORES = 8
B, H, W, C = 16, 512, 512, 3
SPC = B // N_CORES  # samples per core = 2
PX_SAMPLE = H * W  # 262144
P = 128
T = PX_SAMPLE // P  # 2048 pixels per partition; one tile per sample
NT = SPC  # 2 tiles per core

HIST = 3 * 64 * 64  # 12288
HID = 256
MOUT = 27
KT = HIST // P  # 96 k-tiles
GP = 2  # k-tiles packed per PE matmul (lhsT [128, 4], rhs [128, 512])
NGRP = KT // GP  # 24 matmul groups
CH_G = 2  # groups per W1 chunk DMA (2KB/partition contiguous)
NCH = NGRP // CH_G  # 12 chunks

F16 = mybir.dt.float16
F32 = mybir.dt.float32
MULT = mybir.AluOpType.mult
ADD = mybir.AluOpType.add
AF = mybir.ActivationFunctionType

_CACHE = {}


def _build():
    nc = bacc.Bacc(
        "TRN2", target_bir_lowering=False, debug=False, num_devices=N_CORES
    )

    # planar pixel input [tile(=sample), ch, part, T]
    x_d = nc.dram_tensor("x_core", [NT, C, P, T], F32, kind="ExternalInput")
    # histogram for this core's 2 samples, packed [k2, gg*(2*GP) + 2j + s]
    hp_d = nc.dram_tensor("h_packed", [P, KT * SPC], F32, kind="ExternalInput")
    # full W1, host-cast fp16, group-packed [k2, gg, 256j + n]
    w1_d = nc.dram_tensor("w1h", [P, NGRP, GP * HID], F16, kind="ExternalInput")
    b1_d = nc.dram_tensor("b1_rep", [SPC, HID], F32, kind="ExternalInput")
    w2_d = nc.dram_tensor("w2p", [HID // P, P, MOUT], F32, kind="ExternalInput")
    b2_d = nc.dram_tensor("b2bc", [P, SPC * MOUT], F32, kind="ExternalInput")
    eye_d = nc.dram_tensor("eye2", [SPC, SPC], F32, kind="ExternalInput")
    y_d = nc.dram_tensor("y_core", [NT, C, P, T], F16, kind="ExternalOutput")

    MT = HID // P  # 2

    with tile.TileContext(nc) as tc:
        with (
            tc.tile_pool(name="mlp", bufs=1) as mlp_pool,
            tc.tile_pool(name="w1s", bufs=2) as w1_pool,
            tc.tile_pool(name="xin", bufs=2) as x_pool,
            tc.tile_pool(name="pla", bufs=2) as pa_pool,
            tc.tile_pool(name="plb", bufs=1) as pb_pool,
            tc.tile_pool(name="yout", bufs=1) as y_pool,
            tc.tile_pool(name="ps", bufs=1, space="PSUM") as psum_pool,
        ):
            # ---------------- MLP ----------------
            hp_sb = mlp_pool.tile([P, KT * SPC], F16, tag="hp", name="hp")
            nc.gpsimd.dma_start(out=hp_sb, in_=hp_d[:, :])

            b1_sb = mlp_pool.tile([SPC, HID], F32, tag="b1", name="b1")
            nc.scalar.dma_start(out=b1_sb, in_=b1_d[:, :])
            w2_sb = mlp_pool.tile([P, MT, MOUT], F32, tag="w2", name="w2")
            nc.scalar.dma_start(out=w2_sb, in_=w2_d.rearrange("m p n -> p m n"))
            b2_sb = mlp_pool.tile([P, SPC * MOUT], F32, tag="b2", name="b2")
            nc.scalar.dma_start(out=b2_sb, in_=b2_d[:, :])
            eye_sb = mlp_pool.tile([SPC, SPC], F32, tag="eye", name="eye")
            nc.scalar.dma_start(out=eye_sb, in_=eye_d[:, :])

            # 4-packed accumulating matmuls: lhsT [128, 8] covers 4 k-tiles x
            # 2 samples; rhs [128, 1024] = the 4 k-tiles' W1 side by side.
            # psum row 2j+s, cols [256j, 256j+256) holds sample s's partial
            # from k-tile subset j (other cells accumulate don't-care data).
            feat_ps = psum_pool.tile([SPC * GP, GP * HID], F32, tag="featps", name="featps")
            with tc.high_priority():
                for ci in range(NCH):
                    w1c = w1_pool.tile(
                        [P, CH_G, GP * HID], F16, tag="w1c", name=f"w1c{ci}"
                    )
                    nc.sync.dma_start(
                        out=w1c, in_=w1_d[:, ci * CH_G : (ci + 1) * CH_G, :]
                    )
                    for gi in range(CH_G):
                        gg = ci * CH_G + gi
                        nc.tensor.matmul(
                            feat_ps,
                            hp_sb[:, gg * SPC * GP : (gg + 1) * SPC * GP],
                            w1c[:, gi, :],
                            start=(gg == 0),
                            stop=(gg == NGRP - 1),
                        )
            # fold the GP k-tile subsets: [4, 512] -> [2, 256].  Engine reads
            # must start at 32-aligned partitions, so stage the psum in SBUF
            # and move the odd half down with a small SBUF->SBUF DMA.
            u0 = mlp_pool.tile([SPC * GP, GP * HID], F32, tag="u0", name="u0")
            nc.vector.tensor_copy(u0, feat_ps)
            uB = mlp_pool.tile([SPC, HID], F32, tag="uB", name="uB")
            nc.sync.dma_start(out=uB, in_=u0[SPC : 2 * SPC, HID : 2 * HID])
            u2 = mlp_pool.tile([SPC, HID], F32, tag="u2", name="u2")
            nc.vector.tensor_add(u2, u0[0:SPC, 0:HID], uB)
            feat_b = mlp_pool.tile([SPC, HID], F32, tag="featb", name="featb")
            nc.vector.tensor_add(feat_b, u2, b1_sb)
            feat_r = mlp_pool.tile([SPC, HID], F32, tag="featr", name="featr")
            nc.vector.tensor_scalar(feat_r, feat_b, 0.0, None, mybir.AluOpType.max)

            featT_sb = []
            for mt in range(MT):
                ft_ps = psum_pool.tile([P, SPC], F32, tag=f"ftps{mt}", name=f"ftps{mt}")
                nc.tensor.transpose(ft_ps, feat_r[:, mt * P : (mt + 1) * P], eye_sb)
                ft_sb = mlp_pool.tile([P, SPC], F32, tag=f"ft{mt}", name=f"ft{mt}")
                nc.vector.tensor_copy(ft_sb, ft_ps)
                featT_sb.append(ft_sb)

            mb_ps = psum_pool.tile([P, SPC * MOUT], F32, tag="mbps", name="mbps")
            for s in range(SPC):
                for mt in range(MT):
                    nc.tensor.matmul(
                        mb_ps[:, s * MOUT : (s + 1) * MOUT],
                        featT_sb[mt][:, s : s + 1].broadcast_to([P, P]),
                        w2_sb[:, mt, :],
                        start=(mt == 0),
                        stop=(mt == MT - 1),
                    )
            mscal = mlp_pool.tile([P, SPC * MOUT], F32, tag="mscal", name="mscal")
            nc.vector.tensor_add(mscal, mb_ps, b2_sb)

            # ---------------- pixel path (Horner) ----------------
            for t in range(NT):
                def ms(k, c, s=t):
                    j = s * MOUT + 3 * k + c
                    return mscal[:, j : j + 1]

                xt = x_pool.tile([P, C, T], F16, tag="xt", name=f"xt{t}")
                for c in range(C):
                    nc.gpsimd.dma_start(out=xt[:, c, :], in_=x_d[t, c])
                R, G, Bp = xt[:, 0, :], xt[:, 1, :], xt[:, 2, :]
                Rw = xt[:, 0:1, :].broadcast_to([P, C, T])
                Gw = xt[:, 1:2, :].broadcast_to([P, C, T])
                Bw = xt[:, 2:3, :].broadcast_to([P, C, T])

                ysb = y_pool.tile([P, C, T], F16, tag="ysb", name=f"ysb{t}")

                # per-channel scalar products into channel slices of wide
                # tiles.  DVE (4x tensor_scalar): a1 = a3*R + a0,
                # b1 = a4*G + a1, cc = a5*B + a2; ACT: the pure muls,
                # role-grouped so the wide combines unblock early.
                a1w = pa_pool.tile([P, C, T], F16, tag="a1w", name=f"a1w{t}")
                a2w = pa_pool.tile([P, C, T], F16, tag="a2w", name=f"a2w{t}")
                a3w = pa_pool.tile([P, C, T], F16, tag="a3w", name=f"a3w{t}")
                b1w = pa_pool.tile([P, C, T], F16, tag="b1w", name=f"b1w{t}")
                b2w = pb_pool.tile([P, C, T], F16, tag="b2w", name=f"b2w{t}")
                ccw = pb_pool.tile([P, C, T], F16, tag="ccw", name=f"ccw{t}")
                for c in range(C):
                    nc.vector.tensor_scalar(
                        a1w[:, c, :], R, ms(3, c), ms(0, c), MULT, ADD
                    )
                for c in range(C):
                    nc.vector.tensor_scalar(
                        b1w[:, c, :], G, ms(4, c), ms(1, c), MULT, ADD
                    )
                for c in range(C):
                    nc.scalar.mul(a2w[:, c, :], G, ms(6, c))
                for c in range(C):
                    nc.scalar.mul(a3w[:, c, :], Bp, ms(7, c))
                for c in range(C):
                    nc.scalar.activation(
                        ccw[:, c, :], Bp, AF.Identity, bias=ms(2, c), scale=ms(5, c)
                    )
                for c in range(C):
                    nc.scalar.mul(b2w[:, c, :], Bp, ms(8, c))

                # wide channel-merged combines on DVE
                a12 = pb_pool.tile([P, C, T], F16, tag="a12", name=f"a12{t}")
                nc.vector.tensor_add(a12, a1w, a2w)
                aa = pb_pool.tile([P, C, T], F16, tag="aa", name=f"aa{t}")
                nc.vector.tensor_add(aa, a12, a3w)
                ra = pa_pool.tile([P, C, T], F16, tag="a2w", name=f"ra{t}")
                nc.vector.tensor_mul(ra, Rw, aa)
                bc = pa_pool.tile([P, C, T], F16, tag="a1w", name=f"bc{t}")
                nc.vector.tensor_mul(bc, Bw, ccw)
                bb = pb_pool.tile([P, C, T], F16, tag="bb", name=f"bb{t}")
                nc.vector.tensor_add(bb, b1w, b2w)
                gb = pa_pool.tile([P, C, T], F16, tag="a3w", name=f"gb{t}")
                nc.vector.tensor_mul(gb, Gw, bb)
                y1 = pa_pool.tile([P, C, T], F16, tag="b1w", name=f"y1_{t}")
                nc.vector.tensor_add(y1, ra, gb)
                nc.vector.tensor_add(ysb, y1, bc)

                nc.sync.dma_start(out=y_d[t].rearrange("c p j -> p c j"), in_=ysb)

    nc.compile()
    return nc


def _prep_inputs(x, histogram, W1, b1, W2, b2):
    """Host-side sharding / layout packing.  The only host dtype change is
    W1 fp32->fp16 (identical values to what the device cast DMA would
    produce; halves the streamed bytes)."""
    x = np.asarray(x, dtype=np.float32)
    hist = np.asarray(histogram, dtype=np.float32).reshape(B, HIST)
    W1 = np.asarray(W1, dtype=np.float32)
    b1 = np.asarray(b1, dtype=np.float32)
    W2 = np.asarray(W2, dtype=np.float32)
    b2 = np.asarray(b2, dtype=np.float32)

    # [k, n] -> [k2, gg, 256j + n] fp16 (4 k-tiles packed side by side)
    w1h = np.ascontiguousarray(
        W1.reshape(NGRP, GP, P, HID)
        .transpose(2, 0, 1, 3)
        .reshape(P, NGRP, GP * HID)
        .astype(np.float16)
    )
    w2p = np.ascontiguousarray(W2.reshape(HID // P, P, MOUT))
    b1rep = np.ascontiguousarray(np.broadcast_to(b1, (SPC, HID)))
    b2bc = np.ascontiguousarray(np.broadcast_to(np.tile(b2, SPC), (P, SPC * MOUT)))
    eye2 = np.eye(SPC, dtype=np.float32)

    in_maps = []
    for core in range(N_CORES):
        # pixels of sample s: [px, ch] -> [ch, p, j], px = p*T + j
        xs = x[core * SPC : (core + 1) * SPC].reshape(SPC, P, T, C)
        x_core = np.ascontiguousarray(xs.transpose(0, 3, 1, 2))
        hs = hist[core * SPC : (core + 1) * SPC]  # [2, HIST]
        # hp[k2, gg*8 + 2j + s] = h[s, (4gg+j)*128 + k2]
        hp = np.ascontiguousarray(
            hs.reshape(SPC, NGRP, GP, P)
            .transpose(3, 1, 2, 0)
            .reshape(P, KT * SPC)
        )
        in_maps.append(
            {
                "x_core": x_core,
                "h_packed": hp,
                "w1h": w1h,
                "b1_rep": b1rep,
                "w2p": w2p,
                "b2bc": b2bc,
                "eye2": eye2,
            }
        )
    return in_maps


def run(trace=False, **inputs):
    if "nc" not in _CACHE:
        _CACHE["nc"] = _build()
    nc = _CACHE["nc"]
    in_maps = _prep_inputs(**inputs)
    res = bass_utils.run_bass_kernel_spmd(
        nc, in_maps, core_ids=list(range(N_CORES)), trace=trace
    )
    outs = np.stack([r["y_core"] for r in res.results])  # [8, NT, C, P, T] f16
    # [core, s, c, p, j] -> [B, H, W, C]
    y = (
        outs.reshape(N_CORES * SPC, C, P * T)
        .transpose(0, 2, 1)
        .reshape(B, H, W, C)
        .astype(np.float32)
    )
    return y, res


def kernel(**inputs) -> np.ndarray:
    y, _ = run(trace=False, **inputs)
    return y


if __name__ == "__main__":
    rng = np.random.default_rng(0)
    ins = {
        "x": rng.random((B, H, W, C), dtype=np.float32),
        "histogram": rng.random((B, 3, 64, 64), dtype=np.float32),
        "W1": (rng.standard_normal((HIST, HID)) / np.sqrt(HIST)).astype(np.float32),
        "b1": np.zeros(HID, np.float32),
        "W2": (rng.standard_normal((HID, MOUT)) / np.sqrt(HID)).astype(np.float32),
        "b2": np.zeros(MOUT, np.float32),
    }
    y = kernel(**ins)
    print("out", y.shape, y.dtype, float(np.abs(y).max()))


# revision 19
# speedup vs baseline: 1.3061x; 1.1737x over previous
"""AWBNet (wo R2) Trainium2 kernel, v2.

Math (per sample b):
  m = reshape(relu(hist_flat @ W1 + b1) @ W2 + b2, [9, 3])
  y[px, c] = m0c r + m1c g + m2c b + m3c r^2 + m4c g^2 + m5c b^2
           + m6c rg + m7c rb + m8c gb

Device strategy (8 cores, pure data parallel, 2 samples/core):
  * MLP: the full W1 is streamed per core as fp16 (host-cast; the device
    DMA would cast to fp16 anyway, this just halves the HBM read) on the
    sync HWDGE ring, in chunks pipelined with the 96 accumulating PE
    matmuls (lhsT = packed histogram slices [128, 2]).  feat -> relu ->
    PE transpose -> stride-0-broadcast W2 matmul produce mscal[P, 54]
    fp32 coefficients replicated across partitions.
  * Pixels: one tile per sample, [128, 2048] planar fp16 planes loaded
    via SWDGE cast DMAs (host pre-packs x planar, so there is no on-device
    deinterleave).  Per-pixel evaluation uses the Horner form
        y_c = R*(a0 + a3 R + a6 G + a7 B) + G*(a1 + a4 G + a8 B)
            + B*(a2 + a5 B)
    with the per-channel scalar products on ACT (scale/bias activations)
    and DVE (4x-mode tensor_scalar), and all tensor-tensor combines as
    channel-merged wide [128, 3, 2048] DVE ops (2x fp16 mode).  The Pool
    engine is deliberately compute-free: its Q7 tensor ops are slow and
    degrade concurrent DVE throughput (measured), so it only issues the
    SWDGE cast DMAs.
  * y is produced as fp16 planes and stored fp16 (half the write
    traffic); the host casts back to fp32 on assembly.
"""

import sys

import numpy as np

for _p in ("/opt/trn_rl_repo",):
    if _p not in sys.path:
        sys.path.insert(0, _p)

import concourse.bacc as bacc
import concourse.mybir as mybir
import concourse.tile as tile
from concourse import bass_utils

# ---- problem constants (hardcoded per contract) ----
N_CORES = 8
B, H, W, C = 16, 512, 512, 3
SPC = B // N_CORES  # samples per core = 2
PX_SAMPLE = H * W  # 262144
P = 128
T = PX_SAMPLE // P  # 2048 pixels per partition; one tile per sample
NT = SPC  # 2 tiles per core

HIST = 3 * 64 * 64  # 12288
HID = 256
MOUT = 27
KT = HIST // P  # 96 k-tiles
GP = 2  # k-tiles packed per PE matmul (lhsT [128, 4], rhs [128, 512])
NGRP = KT // GP  # 24 matmul groups
CH_G = 1  # groups per W1 chunk DMA (1KB/partition contiguous)
NCH = NGRP // CH_G  # 12 chunks

F16 = mybir.dt.float16
F32 = mybir.dt.float32
MULT = mybir.AluOpType.mult
ADD = mybir.AluOpType.add
AF = mybir.ActivationFunctionType

_CACHE = {}


def _build():
    nc = bacc.Bacc(
        "TRN2", target_bir_lowering=False, debug=False, num_devices=N_CORES
    )

    # planar pixel input [tile(=sample), ch, part, T]
    x_d = nc.dram_tensor("x_core", [NT, C, P, T], F32, kind="ExternalInput")
    # histogram for this core's 2 samples, packed [k2, gg*(2*GP) + 2j + s]
    hp_d = nc.dram_tensor("h_packed", [P, KT * SPC], F32, kind="ExternalInput")
    # full W1, host-cast fp16, group-packed [k2, gg, 256j + n]
    w1_d = nc.dram_tensor("w1h", [P, NGRP, GP * HID], F16, kind="ExternalInput")
    b1_d = nc.dram_tensor("b1_rep", [SPC, HID], F32, kind="ExternalInput")
    w2_d = nc.dram_tensor("w2p", [HID // P, P, MOUT], F32, kind="ExternalInput")
    b2_d = nc.dram_tensor("b2bc", [P, SPC * MOUT], F32, kind="ExternalInput")
    eye_d = nc.dram_tensor("eye2", [SPC, SPC], F32, kind="ExternalInput")
    y_d = nc.dram_tensor("y_core", [NT, C, P, T], F16, kind="ExternalOutput")

    MT = HID // P  # 2

    with tile.TileContext(nc) as tc:
        with (
            tc.tile_pool(name="mlp", bufs=1) as mlp_pool,
            tc.tile_pool(name="w1s", bufs=7) as w1_pool,
            tc.tile_pool(name="xin", bufs=2) as x_pool,
            tc.tile_pool(name="pla", bufs=2) as pa_pool,
            tc.tile_pool(name="plb", bufs=1) as pb_pool,
            tc.tile_pool(name="yout", bufs=1) as y_pool,
            tc.tile_pool(name="ps", bufs=1, space="PSUM") as psum_pool,
        ):
            # ---------------- MLP ----------------
            hp_sb = mlp_pool.tile([P, KT * SPC], F16, tag="hp", name="hp")
            nc.gpsimd.dma_start(out=hp_sb, in_=hp_d[:, :])

            b1_sb = mlp_pool.tile([SPC, HID], F32, tag="b1", name="b1")
            nc.scalar.dma_start(out=b1_sb, in_=b1_d[:, :])
            w2_sb = mlp_pool.tile([P, MT, MOUT], F32, tag="w2", name="w2")
            nc.scalar.dma_start(out=w2_sb, in_=w2_d.rearrange("m p n -> p m n"))
            b2_sb = mlp_pool.tile([P, SPC * MOUT], F32, tag="b2", name="b2")
            nc.scalar.dma_start(out=b2_sb, in_=b2_d[:, :])
            eye_sb = mlp_pool.tile([SPC, SPC], F32, tag="eye", name="eye")
            nc.scalar.dma_start(out=eye_sb, in_=eye_d[:, :])

            # 4-packed accumulating matmuls: lhsT [128, 8] covers 4 k-tiles x
            # 2 samples; rhs [128, 1024] = the 4 k-tiles' W1 side by side.
            # psum row 2j+s, cols [256j, 256j+256) holds sample s's partial
            # from k-tile subset j (other cells accumulate don't-care data).
            feat_ps = psum_pool.tile([SPC * GP, GP * HID], F32, tag="featps", name="featps")
            with tc.high_priority():
                for ci in range(NCH):
                    w1c = w1_pool.tile(
                        [P, CH_G, GP * HID], F16, tag="w1c", name=f"w1c{ci}"
                    )
                    nc.sync.dma_start(
                        out=w1c, in_=w1_d[:, ci * CH_G : (ci + 1) * CH_G, :]
                    )
                    for gi in range(CH_G):
                        gg = ci * CH_G + gi
                        nc.tensor.matmul(
                            feat_ps,
                            hp_sb[:, gg * SPC * GP : (gg + 1) * SPC * GP],
                            w1c[:, gi, :],
                            start=(gg == 0),
                            stop=(gg == NGRP - 1),
                        )
            # fold the GP k-tile subsets: [4, 512] -> [2, 256].  Engine reads
            # must start at 32-aligned partitions, so stage the psum in SBUF
            # and move the odd half down with a small SBUF->SBUF DMA.
            u0 = mlp_pool.tile([SPC * GP, GP * HID], F32, tag="u0", name="u0")
            nc.vector.tensor_copy(u0, feat_ps)
            uB = mlp_pool.tile([SPC, HID], F32, tag="uB", name="uB")
            nc.sync.dma_start(out=uB, in_=u0[SPC : 2 * SPC, HID : 2 * HID])
            u2 = mlp_pool.tile([SPC, HID], F32, tag="u2", name="u2")
            nc.vector.tensor_add(u2, u0[0:SPC, 0:HID], uB)
            feat_b = mlp_pool.tile([SPC, HID], F32, tag="featb", name="featb")
            nc.vector.tensor_add(feat_b, u2, b1_sb)
            feat_r = mlp_pool.tile([SPC, HID], F32, tag="featr", name="featr")
            nc.vector.tensor_scalar(feat_r, feat_b, 0.0, None, mybir.AluOpType.max)

            featT_sb = []
            for mt in range(MT):
                ft_ps = psum_pool.tile([P, SPC], F32, tag=f"ftps{mt}", name=f"ftps{mt}")
                nc.tensor.transpose(ft_ps, feat_r[:, mt * P : (mt + 1) * P], eye_sb)
                ft_sb = mlp_pool.tile([P, SPC], F32, tag=f"ft{mt}", name=f"ft{mt}")
                nc.vector.tensor_copy(ft_sb, ft_ps)
                featT_sb.append(ft_sb)

            mb_ps = psum_pool.tile([P, SPC * MOUT], F32, tag="mbps", name="mbps")
            for s in range(SPC):
                for mt in range(MT):
                    nc.tensor.matmul(
                        mb_ps[:, s * MOUT : (s + 1) * MOUT],
                        featT_sb[mt][:, s : s + 1].broadcast_to([P, P]),
                        w2_sb[:, mt, :],
                        start=(mt == 0),
                        stop=(mt == MT - 1),
                    )
            mscal = mlp_pool.tile([P, SPC * MOUT], F32, tag="mscal", name="mscal")
            nc.vector.tensor_add(mscal, mb_ps, b2_sb)

            # ---------------- pixel path (Horner) ----------------
            for t in range(NT):
                def ms(k, c, s=t):
                    j = s * MOUT + 3 * k + c
                    return mscal[:, j : j + 1]

                xt = x_pool.tile([P, C, T], F16, tag="xt", name=f"xt{t}")
                for c in range(C):
                    nc.gpsimd.dma_start(out=xt[:, c, :], in_=x_d[t, c])
                R, G, Bp = xt[:, 0, :], xt[:, 1, :], xt[:, 2, :]
                Rw = xt[:, 0:1, :].broadcast_to([P, C, T])
                Gw = xt[:, 1:2, :].broadcast_to([P, C, T])
                Bw = xt[:, 2:3, :].broadcast_to([P, C, T])

                ysb = y_pool.tile([P, C, T], F16, tag="ysb", name=f"ysb{t}")

                # per-channel scalar products into channel slices of wide
                # tiles.  DVE (4x tensor_scalar): a1 = a3*R + a0,
                # b1 = a4*G + a1, cc = a5*B + a2; ACT: the pure muls,
                # role-grouped so the wide combines unblock early.
                a1w = pa_pool.tile([P, C, T], F16, tag="a1w", name=f"a1w{t}")
                a2w = pa_pool.tile([P, C, T], F16, tag="a2w", name=f"a2w{t}")
                a3w = pa_pool.tile([P, C, T], F16, tag="a3w", name=f"a3w{t}")
                b1w = pa_pool.tile([P, C, T], F16, tag="b1w", name=f"b1w{t}")
                b2w = pb_pool.tile([P, C, T], F16, tag="b2w", name=f"b2w{t}")
                ccw = pb_pool.tile([P, C, T], F16, tag="ccw", name=f"ccw{t}")
                for c in range(C):
                    nc.vector.tensor_scalar(
                        a1w[:, c, :], R, ms(3, c), ms(0, c), MULT, ADD
                    )
                for c in range(C):
                    nc.vector.tensor_scalar(
                        b1w[:, c, :], G, ms(4, c), ms(1, c), MULT, ADD
                    )
                for c in range(C):
                    nc.scalar.mul(a2w[:, c, :], G, ms(6, c))
                for c in range(C):
                    nc.scalar.mul(a3w[:, c, :], Bp, ms(7, c))
                for c in range(C):
                    nc.scalar.activation(
                        ccw[:, c, :], Bp, AF.Identity, bias=ms(2, c), scale=ms(5, c)
                    )
                for c in range(C):
                    nc.scalar.mul(b2w[:, c, :], Bp, ms(8, c))

                # wide channel-merged combines on DVE
                a12 = pb_pool.tile([P, C, T], F16, tag="a12", name=f"a12{t}")
                nc.vector.tensor_add(a12, a1w, a2w)
                aa = pb_pool.tile([P, C, T], F16, tag="aa", name=f"aa{t}")
                nc.vector.tensor_add(aa, a12, a3w)
                ra = pa_pool.tile([P, C, T], F16, tag="a2w", name=f"ra{t}")
                nc.vector.tensor_mul(ra, Rw, aa)
                bc = pa_pool.tile([P, C, T], F16, tag="a1w", name=f"bc{t}")
                nc.vector.tensor_mul(bc, Bw, ccw)
                bb = pb_pool.tile([P, C, T], F16, tag="bb", name=f"bb{t}")
                nc.vector.tensor_add(bb, b1w, b2w)
                gb = pa_pool.tile([P, C, T], F16, tag="a3w", name=f"gb{t}")
                nc.vector.tensor_mul(gb, Gw, bb)
                y1 = pa_pool.tile([P, C, T], F16, tag="b1w", name=f"y1_{t}")
                nc.vector.tensor_add(y1, ra, gb)
                nc.vector.tensor_add(ysb, y1, bc)

                nc.sync.dma_start(out=y_d[t].rearrange("c p j -> p c j"), in_=ysb)

    nc.compile()
    return nc


def _prep_inputs(x, histogram, W1, b1, W2, b2):
    """Host-side sharding / layout packing.  The only host dtype change is
    W1 fp32->fp16 (identical values to what the device cast DMA would
    produce; halves the streamed bytes)."""
    x = np.asarray(x, dtype=np.float32)
    hist = np.asarray(histogram, dtype=np.float32).reshape(B, HIST)
    W1 = np.asarray(W1, dtype=np.float32)
    b1 = np.asarray(b1, dtype=np.float32)
    W2 = np.asarray(W2, dtype=np.float32)
    b2 = np.asarray(b2, dtype=np.float32)

    # [k, n] -> [k2, gg, 256j + n] fp16 (4 k-tiles packed side by side)
    w1h = np.ascontiguousarray(
        W1.reshape(NGRP, GP, P, HID)
        .transpose(2, 0, 1, 3)
        .reshape(P, NGRP, GP * HID)
        .astype(np.float16)
    )
    w2p = np.ascontiguousarray(W2.reshape(HID // P, P, MOUT))
    b1rep = np.ascontiguousarray(np.broadcast_to(b1, (SPC, HID)))
    b2bc = np.ascontiguousarray(np.broadcast_to(np.tile(b2, SPC), (P, SPC * MOUT)))
    eye2 = np.eye(SPC, dtype=np.float32)

    in_maps = []
    for core in range(N_CORES):
        # pixels of sample s: [px, ch] -> [ch, p, j], px = p*T + j
        xs = x[core * SPC : (core + 1) * SPC].reshape(SPC, P, T, C)
        x_core = np.ascontiguousarray(xs.transpose(0, 3, 1, 2))
        hs = hist[core * SPC : (core + 1) * SPC]  # [2, HIST]
        # hp[k2, gg*8 + 2j + s] = h[s, (4gg+j)*128 + k2]
        hp = np.ascontiguousarray(
            hs.reshape(SPC, NGRP, GP, P)
            .transpose(3, 1, 2, 0)
            .reshape(P, KT * SPC)
        )
        in_maps.append(
            {
                "x_core": x_core,
                "h_packed": hp,
                "w1h": w1h,
                "b1_rep": b1rep,
                "w2p": w2p,
                "b2bc": b2bc,
                "eye2": eye2,
            }
        )
    return in_maps


def run(trace=False, **inputs):
    if "nc" not in _CACHE:
        _CACHE["nc"] = _build()
    nc = _CACHE["nc"]
    in_maps = _prep_inputs(**inputs)
    res = bass_utils.run_bass_kernel_spmd(
        nc, in_maps, core_ids=list(range(N_CORES)), trace=trace
    )
    outs = np.stack([r["y_core"] for r in res.results])  # [8, NT, C, P, T] f16
    # [core, s, c, p, j] -> [B, H, W, C]
    y = (
        outs.reshape(N_CORES * SPC, C, P * T)
        .transpose(0, 2, 1)
        .reshape(B, H, W, C)
        .astype(np.float32)
    )
    return y, res


def kernel(**inputs) -> np.ndarray:
    y, _ = run(trace=False, **inputs)
    return y


if __name__ == "__main__":
    rng = np.random.default_rng(0)
    ins = {
        "x": rng.random((B, H, W, C), dtype=np.float32),
        "histogram": rng.random((B, 3, 64, 64), dtype=np.float32),
        "W1": (rng.standard_normal((HIST, HID)) / np.sqrt(HIST)).astype(np.float32),
        "b1": np.zeros(HID, np.float32),
        "W2": (rng.standard_normal((HID, MOUT)) / np.sqrt(HID)).astype(np.float32),
        "b2": np.zeros(MOUT, np.float32),
    }
    y = kernel(**ins)
    print("out", y.shape, y.dtype, float(np.abs(y).max()))
